# revision 1
# baseline (speedup 1.0000x reference)
"""TRN2 Bass kernel for nn_LogDomainResNet.

The reference network is a signed-log-domain encoding of a plain
real-domain tanh ResNet:

    v0      = sign_x * exp(log_abs_x)
    v_{i+1} = tanh(v_i @ W_i + b_i) + v_i        (7 inner layers)
    t       = v_7 @ W_final
    out     = stack([sign(t), log|t|])

All slog plumbing (per-row max, exp/log per layer) cancels exactly, so the
kernel computes in the real domain. Values stay bounded (|v| < 20), so fp32
range is never an issue.

Precision: each matmul is one fp16 pass (vh@Wh, exact products into fp32
PSUM) plus one fp8e4m3 DoubleRow correction pass computing
(vl*2^12)@(W*2^4) + v@(Wl*2^16) — the first-order error terms of the fp16
quantization of both operands, scaled into fp8 range. Combined as
main + 2^-16*corr this leaves ~2^-15 relative error per layer (CPU-checked
end-to-end and confirmed on HW: rel err 5.8e-3 vs the slog reference,
gate 2e-2). DoubleRow packs the two correction contractions into one PE
pass at 0.5 cyc/row, so a layer costs 1.5 bf16-equivalent passes instead
of the 3 passes a near-fp32 split needs.

The residual stream av stays in fp32 SBUF and is updated in place
(av += tanh-branch); vh (fp16) and vc (fp8 pair: vl*2^12, v) are
requantized from it each layer and feed the PE.

Engine balance: PE runs the matmul streams back-to-back (the fp16 weights
are pre-scaled by 2^16 host-side so the fp8 correction accumulates into
the same PSUM group, and tanh/sign/abs read PSUM directly with a 2^-16
scale — no separate combine pass). ACT does tanh, abs/ln, and the fp8
residual cast; DVE does the in-place residual add, the residual split,
and the fp8 sign compare; Pool (GpSimd) does the fp16/fp8 requantize
copies. Consumer work per tile stays under the PE tile time on every
engine, and a 6-deep PSUM pool (2-deep in the final layer, to cap the
drain tail) keeps the PE from stalling on bank reuse.

Layout: activations live transposed ([feature -> partitions, batch ->
free]); the final layer swaps operands (lhsT = v^T tile) to produce t in
natural [batch, feature] layout, so outputs DMA out contiguously.

Sharding: data-parallel over the batch axis, 1024 rows per core x 8 cores.
"""

import numpy as np

_B, _D, _NL = 8192, 1024, 8  # batch, width, layers (7 inner + final)
_NCORES = 8
_BP = _B // _NCORES          # batch rows per core
_P = 128
_KC = _D // _P               # contraction chunks per matmul
_BT = _BP // _P              # batch tiles (input/final stages)
_BCH = 512                   # PSUM free dim
_NBC = _BP // _BCH           # batch chunks per layer pass
_NT = _D // _P               # out-feature tiles per layer

_cached_nc = None
last_results = None  # BassKernelResults from the most recent run (for test.py)


def _build():
    import concourse.mybir as mybir
    from concourse import bacc
    from concourse.tile import TileContext

    f32, f16, f8 = mybir.dt.float32, mybir.dt.float16, mybir.dt.float8e4
    AF = mybir.ActivationFunctionType
    ALU = mybir.AluOpType
    DR = mybir.MatmulPerfMode.DoubleRow

    nc = bacc.Bacc("TRN2", target_bir_lowering=False, debug=False)
    # inputs arrive pre-transposed ([feature, batch]) and sign as fp8
    d_sgn = nc.dram_tensor("sign_xt", [_D, _BP], f8, kind="ExternalInput")
    d_lab = nc.dram_tensor("log_abs_xt", [_D, _BP], f32, kind="ExternalInput")
    d_wh = nc.dram_tensor("wh", [_NL, _D, _D], f16, kind="ExternalInput")
    # wc[l, k, 0, n] = fp8(W*2^4); wc[l, k, 1, n] = fp8((W - fp16(W))*2^16)
    d_wc = nc.dram_tensor("wc", [_NL, _D, 2, _D], f8, kind="ExternalInput")
    d_bias = nc.dram_tensor("bias", [_P, (_NL - 1) * _NT], f32, kind="ExternalInput")
    d_sgo = nc.dram_tensor("out_sg", [_BP, _D], f8, kind="ExternalOutput")
    d_lgo = nc.dram_tensor("out_lg", [_BP, _D], f16, kind="ExternalOutput")

    with TileContext(nc) as tc:
        with (
            tc.tile_pool(name="const", bufs=1) as constp,
            tc.tile_pool(name="w", bufs=2) as wp,
            tc.tile_pool(name="v", bufs=2) as vp,
            tc.tile_pool(name="av", bufs=1) as avp,
            tc.tile_pool(name="inp", bufs=2) as inp,
            tc.tile_pool(name="tmp", bufs=3) as tmp,
            tc.tile_pool(name="ps", bufs=6, space="PSUM") as ps,
            tc.tile_pool(name="psf", bufs=2, space="PSUM") as psf,
        ):
            # ---- input: v0 = sign * exp(log_abs), already [feature, batch] ----
            av = avp.tile([_P, _KC, _BP], f32, tag="av")  # residual, fp32
            vh = vp.tile([_P, _KC, _BP], f16, tag="vh")
            vc = vp.tile([_P, _KC, 2, _BP], f8, tag="vc")
            lab_r = d_lab.rearrange("(c p) b -> p c b", p=_P)
            sgn_r = d_sgn.rearrange("(c p) b -> p c b", p=_P)
            sgn_sb = inp.tile([_P, _KC, _BP], f8, tag="sgn", bufs=1)
            # phase-ordered emission: engines drain critical ops (exp ->
            # mul -> vh) for every strip before the correction-plane tail,
            # so layer 0 starts as soon as chunk 0's vh lands. DMA order:
            # chunk-0 inputs, layer-0 weights + bias, chunk-1 inputs.
            # PE warm-up: throwaway matmuls on a zeroed tile keep the
            # PE HAM/p-state at full clock through the input-stage idle gap
            warm = constp.tile([_P, _BCH], f16, tag="warm")
            nc.gpsimd.memset(warm[:], 0.0)
            half = constp.tile([_P, _BCH], f32, tag="half")
            nc.gpsimd.memset(half[:], 0.5)
            wps = psf.tile([_P, _BCH], f32, tag="fm")
            for _ in range(32):
                nc.tensor.matmul(
                    wps[:], warm[:, :_P], warm[:], start=True, stop=True,
                )
            strips = [slice(t * _P, (t + 1) * _P) for t in range(_BT)]
            for t, tsl in enumerate(strips):
                if t == _BT // 2:
                    # split the layer-0 fp16 weight DMA so the first
                    # contraction chunks land ~3us earlier
                    whs0 = wp.tile([_P, _KC, _D], f16, tag="wh")
                    wh0_r = d_wh[0].rearrange("(c p) n -> p c n", p=_P)
                    for q in range(4):
                        qsl = slice(q * _KC // 4, (q + 1) * _KC // 4)
                        nc.sync.dma_start(whs0[:, qsl, :], wh0_r[:, qsl, :])
                    wcs0 = wp.tile([_P, _KC, 2, _D], f8, tag="wc")
                    nc.sync.dma_start(
                        wcs0[:],
                        d_wc[0].rearrange("(c p) two n -> p c two n", p=_P),
                    )
                    bias_sb = constp.tile([_P, (_NL - 1) * _NT], f32)
                    nc.sync.dma_start(bias_sb[:], d_bias[:, :])
                nc.sync.dma_start(av[:, :, tsl], lab_r[:, :, tsl])
                if t % (_BT // 2) == 0:
                    csl = slice(t * _P, t * _P + _BCH)
                    nc.sync.dma_start(sgn_sb[:, :, csl], sgn_r[:, :, csl])
            half_n = _BT // 2
            for group in (strips[:half_n], strips[half_n:]):
                for tsl in group:
                    nc.scalar.activation(av[:, :, tsl], av[:, :, tsl], AF.Exp)
                for tsl in group:
                    nc.vector.tensor_mul(
                        out=av[:, :, tsl], in0=av[:, :, tsl],
                        in1=sgn_sb[:, :, tsl],
                    )
                for t, tsl in enumerate(group):
                    eng = nc.vector if t % 2 else nc.gpsimd
                    eng.tensor_copy(out=vh[:, :, tsl], in_=av[:, :, tsl])
                d_ts = []
                for tsl in group:
                    d_t = tmp.tile([_P, _KC, _P], f32, tag="di", bufs=4)
                    d_ts.append(d_t)
                    nc.vector.tensor_sub(
                        out=d_t[:], in0=av[:, :, tsl], in1=vh[:, :, tsl]
                    )
                for t, (tsl, d_t) in enumerate(zip(group, d_ts)):
                    eng = nc.gpsimd if t % 2 else nc.vector
                    eng.tensor_scalar_mul(
                        out=vc[:, :, 0, tsl], in0=d_t[:], scalar1=4096.0
                    )
                for t, tsl in enumerate(group):
                    eng = nc.vector if t % 2 else nc.gpsimd
                    eng.tensor_copy(out=vc[:, :, 1, tsl], in_=vh[:, :, tsl])

            # ---- 7 inner layers: v = tanh(v @ W + b) + v ----
            for i in range(_NL - 1):
                if i == 0:
                    whs, wcs = whs0, wcs0
                else:
                    whs = wp.tile([_P, _KC, _D], f16, tag="wh")
                    nc.sync.dma_start(
                        whs[:], d_wh[i].rearrange("(c p) n -> p c n", p=_P)
                    )
                    wcs = wp.tile([_P, _KC, 2, _D], f8, tag="wc")
                    nc.sync.dma_start(
                        wcs[:], d_wc[i].rearrange("(c p) two n -> p c two n", p=_P)
                    )
                vh_new = vp.tile([_P, _KC, _BP], f16, tag="vh")
                vc_new = vp.tile([_P, _KC, 2, _BP], f8, tag="vc")
                for b0, bw in [(0, _BCH), (_BCH, _BCH)]:
                    bsl = slice(b0, b0 + bw)
                    for n in range(_NT):
                        nsl = slice(n * _P, (n + 1) * _P)
                        # wh is pre-scaled by 2^16 host-side, so the fp16
                        # main pass and the fp8 DR correction land at the
                        # same 2^16 scale and share one PSUM accumulation
                        # group; tanh then reads PSUM with scale=2^-16.
                        pm = ps.tile([_P, bw], f32, tag="mm")
                        for c in range(_KC):
                            nc.tensor.matmul(
                                pm[:], whs[:, c, nsl], vh[:, c, bsl],
                                start=(c == 0), stop=False,
                            )
                        for c in range(_KC):
                            nc.tensor.matmul(
                                pm[:], wcs[:, c, :, nsl], vc[:, c, :, bsl],
                                start=False, stop=(c == _KC - 1),
                                perf_mode=DR,
                            )
                        u = tmp.tile([_P, bw], f32, tag="u")
                        nc.scalar.activation(
                            u[:], pm[:], AF.Tanh, scale=2.0 ** -16,
                            bias=bias_sb[:, i * _NT + n : i * _NT + n + 1],
                        )
                        nc.vector.tensor_add(
                            out=av[:, n, bsl], in0=av[:, n, bsl], in1=u[:]
                        )
                        nc.gpsimd.tensor_copy(
                            out=vh_new[:, n, bsl], in_=av[:, n, bsl]
                        )
                        d_t = tmp.tile([_P, bw], f32, tag="d")
                        nc.vector.tensor_sub(
                            out=d_t[:], in0=av[:, n, bsl], in1=vh_new[:, n, bsl]
                        )
                        nc.scalar.activation(
                            vc_new[:, n, 0, bsl], d_t[:], AF.Copy, scale=4096.0
                        )
                        nc.gpsimd.tensor_copy(
                            out=vc_new[:, n, 1, bsl], in_=vh_new[:, n, bsl]
                        )
                vh, vc = vh_new, vc_new

            # ---- final layer: t = v @ W_f, out = [sign(t), log|t|] ----
            whf = wp.tile([_P, _KC, _D], f16, tag="wh")
            nc.sync.dma_start(
                whf[:], d_wh[_NL - 1].rearrange("(c p) n -> p c n", p=_P)
            )
            wcf = wp.tile([_P, _KC, 2, _D], f8, tag="wc")
            nc.sync.dma_start(
                wcf[:], d_wc[_NL - 1].rearrange("(c p) two n -> p c two n", p=_P)
            )
            for bt in range(_BT):
                bsl = slice(bt * _P, (bt + 1) * _P)
                for j0, jw in [(0, _BCH), (_BCH, _BCH)]:
                    nsl = slice(j0, j0 + jw)
                    pm = psf.tile([_P, jw], f32, tag="fm")
                    for c in range(_KC):
                        nc.tensor.matmul(
                            pm[:], vh[:, c, bsl], whf[:, c, nsl],
                            start=(c == 0), stop=False,
                        )
                    for c in range(_KC):
                        nc.tensor.matmul(
                            pm[:], vc[:, c, :, bsl], wcf[:, c, :, nsl],
                            start=False, stop=(c == _KC - 1),
                            perf_mode=DR,
                        )
                    # sign is invariant to the positive 2^16 scale, so both
                    # outputs read PSUM directly; no separate combine needed.
                    # sign on DVE: (pm >= 0) - 0.5 -> {-0.5, +0.5} in fp8;
                    # the host maps back to +-1 (sign is scale-invariant)
                    sg = tmp.tile([_P, jw], f8, tag="u")
                    nc.vector.scalar_tensor_tensor(
                        out=sg[:], in0=pm[:], scalar=0.0, in1=half[:, :jw],
                        op0=ALU.is_ge, op1=ALU.subtract,
                    )
                    ab = tmp.tile([_P, jw], f32, tag="d")
                    nc.scalar.activation(ab[:], pm[:], AF.Abs, scale=2.0 ** -16)
                    lg = tmp.tile([_P, jw], f16, tag="lg", bufs=2)
                    nc.scalar.activation(lg[:], ab[:], AF.Ln)
                    nc.sync.dma_start(d_sgo[bsl, nsl], sg[:])
                    nc.sync.dma_start(d_lgo[bsl, nsl], lg[:])
    nc.compile()
    return nc


def kernel(sign_x, log_abs_x, inner_kernels, final_kernel):
    global _cached_nc, last_results
    import ml_dtypes
    from concourse.bass_utils import run_bass_kernel_spmd

    if _cached_nc is None:
        _cached_nc = _build()
    nc = _cached_nc

    sign_xt = np.ascontiguousarray(
        np.asarray(sign_x, dtype=np.float32).T.astype(ml_dtypes.float8_e4m3)
    )
    log_abs_xt = np.ascontiguousarray(np.asarray(log_abs_x, dtype=np.float32).T)
    ik = np.asarray(inner_kernels, dtype=np.float32)
    fk = np.asarray(final_kernel, dtype=np.float32)

    W = np.concatenate([ik[:, :_D, :], fk[None]], axis=0)  # [8, 1024, 1024]
    Wh = W.astype(np.float16)
    f8 = ml_dtypes.float8_e4m3
    Wq = (W * 16.0).astype(f8)
    Wl = ((W - Wh.astype(np.float32)) * 65536.0).astype(f8)
    # pre-scale the fp16 main weights so main products match the 2^16
    # correction scale (exact: power of two)
    Wh = (Wh.astype(np.float32) * 65536.0).astype(np.float16)
    wc = np.ascontiguousarray(np.stack([Wq, Wl], axis=2))  # [8, 1024, 2, 1024]
    Wh = np.ascontiguousarray(Wh)
    bias = np.ascontiguousarray(
        ik[:, _D, :].reshape(_NL - 1, _NT, _P).transpose(2, 0, 1).reshape(_P, -1)
    )  # [128, 56]: column (l*8+t) holds bias[l, t*128+p] on partition p

    in_maps = []
    for cid in range(_NCORES):
        sl = slice(cid * _BP, (cid + 1) * _BP)
        in_maps.append({
            "sign_xt": np.ascontiguousarray(sign_xt[:, sl]),
            "log_abs_xt": np.ascontiguousarray(log_abs_xt[:, sl]),
            "wh": Wh,
            "wc": wc,
            "bias": bias,
        })

    last_results = run_bass_kernel_spmd(nc, in_maps, core_ids=list(range(_NCORES)))
    sg = np.sign(np.concatenate(
        [r["out_sg"].astype(np.float32) for r in last_results.results], axis=0
    ))
    lg = np.concatenate(
        [r["out_lg"].astype(np.float32) for r in last_results.results], axis=0
    )
    return np.stack([sg, lg], axis=0)

